# revision 7
# baseline (speedup 1.0000x reference)
"""Gated GCN layer (DDI message passing) on 8 Trainium2 NeuronCores.

Strategy (data-parallel over batch B=256 -> 32 sentences/core):
  - Host: per-sentence edge list -> dense [200,200] adjacency matrix M
    (M[src,dst] += data). Aggregations become dense matmuls:
      in_t  = M^T @ (X W_in + b)   ;  out_t = M @ (X W_out + b)
  - X^T is pre-transposed on host and augmented with a ones-row so the
    biases ride inside the matmul (K = 361).
  - One fused weight matrix wt [361, 1083]:
      [W_in 0:360 | W_out 360:720 | W_loop 720:1080 | gin | gout | gloop]
  - Device per (sentence, row-tile): Z in two PSUM tiles
      zpA [128, 2 banks]: W_in-proj @ bank0, W_out-proj @ bank1
      zpB [128, 1 bank]:  W_loop-proj + the 3 gate columns
    then agg with M/M^T stationary into one 2-bank PSUM tile (gate col
    rides along), and a fused gating epilogue:
      t3 = sigmoid(gloop)*Zloop on eviction (scalar activation scale),
      s1 = (agg_in*sig(gin_agg)) + t3 and s2 = (agg_out*sig)+s1 via
      scalar_tensor_tensor, relu on vector.
  - PE order interleaves main(s) and agg(s-1) at row-tile granularity so
    PSUM-bank reuse never stalls the PE; evictions are spread over
    scalar/vector/gpsimd.
  - Everything bf16 on device (fp32 accumulation in PSUM).
"""
import sys

if "/opt/trn_rl_repo" not in sys.path:
    sys.path.insert(0, "/opt/trn_rl_repo")

from contextlib import ExitStack

import ml_dtypes
import numpy as np

B, NN, EE, DIN, DOUT = 256, 200, 400, 360, 360
NCORES = 8
SPC = B // NCORES          # 32 sentences per core
ROWS = SPC * NN            # 6400 rows per core
KA = DIN + 1               # 361: augmented contraction (ones row for bias)
WCOLS = 3 * DOUT + 3       # 1083 fused weight columns
KCH = [(0, 121), (121, 241), (241, 361)]   # K chunks <= 128
NBF16 = np.dtype(ml_dtypes.bfloat16)

_compiled = None


def _build():
    import concourse.bacc as bacc
    import concourse.mybir as mybir
    from concourse.tile import TileContext

    BF16 = mybir.dt.bfloat16
    F32 = mybir.dt.float32
    AF = mybir.ActivationFunctionType
    OP = mybir.AluOpType

    nc = bacc.Bacc(None, target_bir_lowering=False)
    xt_d = nc.dram_tensor("xt", [KA, ROWS], BF16, kind="ExternalInput")
    wt_d = nc.dram_tensor("wt", [KA, WCOLS], BF16, kind="ExternalInput")
    # per sentence: [mf 0:400 | mb 400:800], each chunked (src rows 0:128 at
    # cols 0:200, src rows 128:200 at cols 200:400; free dim = dst)
    madj_d = nc.dram_tensor("madj", [SPC, 128, 800], BF16, kind="ExternalInput")
    out_d = nc.dram_tensor("out", [SPC, 128, 2 * DOUT], BF16, kind="ExternalOutput")

    with TileContext(nc) as tc, ExitStack() as ctx:
        cpool = ctx.enter_context(tc.tile_pool(name="cpool", bufs=1))
        mpool = ctx.enter_context(tc.tile_pool(name="mpool", bufs=4))
        zspool = ctx.enter_context(tc.tile_pool(name="zspool", bufs=4))
        tpool = ctx.enter_context(tc.tile_pool(name="tpool", bufs=4))
        opool = ctx.enter_context(tc.tile_pool(name="opool", bufs=4))
        zpa = ctx.enter_context(tc.tile_pool(name="zpa", bufs=2, space="PSUM"))
        zpb = ctx.enter_context(tc.tile_pool(name="zpb", bufs=2, space="PSUM"))
        app = ctx.enter_context(tc.tile_pool(name="app", bufs=1, space="PSUM"))

        # ---- resident inputs: weights first (tiny), then X^T with the
        # first sentences' columns landing first ----
        wt_tiles = []
        wt_eng = [nc.scalar, nc.gpsimd, nc.sync]
        for kc, (k0, k1) in enumerate(KCH):
            t = cpool.tile([k1 - k0, WCOLS], BF16, name=f"wt{kc}")
            wt_eng[kc].dma_start(out=t, in_=wt_d[k0:k1, :])
            wt_tiles.append(t)
        xt_tiles = [cpool.tile([k1 - k0, ROWS], BF16, name=f"xt{kc}")
                    for kc, (k0, k1) in enumerate(KCH)]
        first_eng = [nc.scalar, nc.gpsimd, nc.sync]
        for kc, (k0, k1) in enumerate(KCH):
            first_eng[kc].dma_start(out=xt_tiles[kc][:, 0:240], in_=xt_d[k0:k1, 0:240])
        for j in [240, 1040, 1840, 2640, 3440, 4920]:
            w = 800 if j < 3440 else 1480
            for kc, (k0, k1) in enumerate(KCH):
                nc.sync.dma_start(out=xt_tiles[kc][:, j:j + w], in_=xt_d[k0:k1, j:j + w])

        state = {}   # (s, mt) -> per row-tile tensors; s -> madj tile / out tile

        def emit_main(s, mt):
            rows = 128 if mt == 0 else 72
            c0 = s * NN + mt * 128
            if mt == 0:
                madj_t = mpool.tile([128, 800], BF16, tag="madj", name=f"madj{s}")
                nc.gpsimd.dma_start(out=madj_t, in_=madj_d[s])
                state[s] = madj_t
            za = zpa.tile([128, 1024], F32, tag="za", name=f"za{s}_{mt}")
            zb = zpb.tile([128, 512], F32, tag="zb", name=f"zb{s}_{mt}")
            for kc in range(3):
                lt = xt_tiles[kc][:, c0:c0 + rows]
                nc.tensor.matmul(za[0:rows, 0:360], lhsT=lt,
                                 rhs=wt_tiles[kc][:, 0:360],
                                 start=(kc == 0), stop=(kc == 2))
                nc.tensor.matmul(za[0:rows, 512:872], lhsT=lt,
                                 rhs=wt_tiles[kc][:, 360:720],
                                 start=(kc == 0), stop=(kc == 2))
                nc.tensor.matmul(zb[0:rows, 0:363], lhsT=lt,
                                 rhs=wt_tiles[kc][:, 720:1083],
                                 start=(kc == 0), stop=(kc == 2))
            zs_t = zspool.tile([128, 722], BF16, tag="zs", name=f"zs{s}_{mt}")
            sgl = tpool.tile([128, 1], F32, tag="sgl", name=f"sgl{s}_{mt}")
            t3 = tpool.tile([128, 360], BF16, tag="t3", name=f"t3{s}_{mt}")
            # loop gate + gated loop-term eviction (fused scale)
            nc.scalar.activation(sgl[0:rows], zb[0:rows, 362:363], AF.Sigmoid)
            nc.scalar.activation(t3[0:rows], zb[0:rows, 0:360], AF.Copy,
                                 scale=sgl[0:rows, 0:1])
            # evict Z_in/Z_out feats + raw gate cols into agg-rhs layout
            # (gpsimd cannot read PSUM: split across scalar and vector)
            zs_v = zs_t[0:rows].rearrange("p (a c) -> p a c", a=2, c=361)
            nc.scalar.copy(zs_t[0:rows, 0:360], za[0:rows, 0:360])
            nc.vector.tensor_copy(zs_t[0:rows, 361:721], za[0:rows, 512:872])
            nc.scalar.copy(zs_v[:, :, 360], zb[0:rows, 360:362])
            state[(s, mt)] = (zs_t, t3)

        def emit_agg(s, mt):
            rows = 128 if mt == 0 else 72
            d0 = mt * 128
            madj_t = state[s]
            zs_lo, _ = state[(s, 0)]
            zs_hi, _ = state[(s, 1)]
            _, t3 = state[(s, mt)]
            if mt == 1:
                del state[(s, 0)], state[(s, 1)], state[s]
            ap = app.tile([128, 1024], F32, tag="ap", name=f"ap{s}_{mt}")
            # in-arcs: lhsT = mf columns (dst slice), rhs = [Z_in|gin]
            nc.tensor.matmul(ap[0:rows, 0:361],
                             lhsT=madj_t[0:128, d0:d0 + rows],
                             rhs=zs_lo[0:128, 0:361], start=True, stop=False)
            nc.tensor.matmul(ap[0:rows, 0:361],
                             lhsT=madj_t[0:72, 200 + d0:200 + d0 + rows],
                             rhs=zs_hi[0:72, 0:361], start=False, stop=True)
            # out-arcs: lhsT = mb columns, rhs = [Z_out|gout]
            nc.tensor.matmul(ap[0:rows, 512:873],
                             lhsT=madj_t[0:128, 400 + d0:400 + d0 + rows],
                             rhs=zs_lo[0:128, 361:722], start=True, stop=False)
            nc.tensor.matmul(ap[0:rows, 512:873],
                             lhsT=madj_t[0:72, 600 + d0:600 + d0 + rows],
                             rhs=zs_hi[0:72, 361:722], start=False, stop=True)
            # gating epilogue
            sg2 = tpool.tile([128, 2], F32, tag="sg2", name=f"sg2{s}_{mt}")
            ap_v = ap[0:rows].rearrange("p (a c) -> p a c", a=2, c=512)
            nc.scalar.activation(sg2[0:rows], ap_v[:, :, 360], AF.Sigmoid)
            s1 = tpool.tile([128, 360], BF16, tag="s1", name=f"s1{s}_{mt}")
            s2 = tpool.tile([128, 360], BF16, tag="s2", name=f"s2{s}_{mt}")
            nc.vector.scalar_tensor_tensor(
                s1[0:rows], ap[0:rows, 0:360], sg2[0:rows, 0:1], t3[0:rows],
                OP.mult, OP.add)
            nc.vector.scalar_tensor_tensor(
                s2[0:rows], ap[0:rows, 512:872], sg2[0:rows, 1:2], s1[0:rows],
                OP.mult, OP.add)
            ot = opool.tile([128, DOUT], BF16, tag="ot", name=f"ot{s}_{mt}")
            nc.gpsimd.tensor_scalar_max(ot[0:rows], s2[0:rows], 0.0)
            eng = nc.sync if s == SPC - 1 else nc.gpsimd
            eng.dma_start(out=out_d[s, 0:rows, mt * DOUT:(mt + 1) * DOUT],
                          in_=ot[0:rows])

        # software pipeline: agg runs one sentence behind main, interleaved
        # at row-tile granularity
        for s in range(SPC):
            emit_main(s, 0)
            if s > 0:
                emit_agg(s - 1, 0)
            emit_main(s, 1)
            if s > 0:
                emit_agg(s - 1, 1)
        emit_agg(SPC - 1, 0)
        emit_agg(SPC - 1, 1)

    nc.compile()
    return nc


def _get_compiled():
    global _compiled
    if _compiled is None:
        _compiled = _build()
    return _compiled


def kernel(gcn_in, adj_ind, adj_data, w_in, b_in, w_out, b_out, w_loop,
           w_gin, b_gin, w_gout, b_gout, w_gloop):
    from concourse.bass_utils import run_bass_kernel_spmd

    x = np.asarray(gcn_in, np.float32)           # [B, N, DIN]
    idx = np.asarray(adj_ind)[0]                 # [B, E, 2] int
    dat = np.asarray(adj_data, np.float32)[0]    # [B, E]

    # fused weight matrix with bias row:
    # [W_in | W_out | W_loop | gin | gout | gloop]
    wt = np.zeros((KA, WCOLS), np.float32)
    wt[0:DIN, 0:360] = np.asarray(w_in, np.float32)
    wt[DIN, 0:360] = np.asarray(b_in, np.float32)[0]
    wt[0:DIN, 360:720] = np.asarray(w_out, np.float32)
    wt[DIN, 360:720] = np.asarray(b_out, np.float32)[0]
    wt[0:DIN, 720:1080] = np.asarray(w_loop, np.float32)
    wt[0:DIN, 1080] = np.asarray(w_gin, np.float32)[:, 0]
    wt[DIN, 1080] = np.asarray(b_gin, np.float32)[0]
    wt[0:DIN, 1081] = np.asarray(w_gout, np.float32)[:, 0]
    wt[DIN, 1081] = np.asarray(b_gout, np.float32)[0]
    wt[0:DIN, 1082] = np.asarray(w_gloop, np.float32)[:, 0]
    wt = wt.astype(NBF16)

    # dense per-sentence adjacency matrices
    M = np.zeros((B, NN, NN), np.float32)
    bi = np.broadcast_to(np.arange(B)[:, None], idx.shape[:2])
    np.add.at(M, (bi, idx[:, :, 0].astype(np.int64), idx[:, :, 1].astype(np.int64)), dat)

    def chunked(mm):      # [SPC,200,200] -> [SPC,128,400]: two 128-row chunks side by side
        out = np.zeros((SPC, 128, 2 * NN), np.float32)
        out[:, :, 0:NN] = mm[:, 0:128, :]
        out[:, 0:72, NN:2 * NN] = mm[:, 128:200, :]
        return out

    nc = _get_compiled()
    in_maps = []
    for c in range(NCORES):
        xc = x[c * SPC:(c + 1) * SPC].reshape(ROWS, DIN)
        xt = np.empty((KA, ROWS), np.float32)
        xt[0:DIN] = xc.T
        xt[DIN] = 1.0
        mc = M[c * SPC:(c + 1) * SPC]
        madj = np.concatenate(
            [chunked(mc), chunked(np.ascontiguousarray(mc.transpose(0, 2, 1)))],
            axis=2)
        in_maps.append({
            "xt": np.ascontiguousarray(xt).astype(NBF16),
            "wt": wt,
            "madj": madj.astype(NBF16),
        })

    res = run_bass_kernel_spmd(nc, in_maps, core_ids=list(range(NCORES)))
    kernel.last_results = res
    out = np.empty((B, NN, DOUT), np.float32)
    for c in range(NCORES):
        oc = res.results[c]["out"].astype(np.float32)   # [SPC,128,720]
        oc_s = out[c * SPC:(c + 1) * SPC]               # [SPC,200,360]
        oc_s[:, 0:128, :] = oc[:, :, 0:DOUT]
        oc_s[:, 128:200, :] = oc[:, 0:72, DOUT:2 * DOUT]
    return out


# revision 8
# speedup vs baseline: 2.0824x; 2.0824x over previous
"""Gated GCN layer (DDI message passing) on 8 Trainium2 NeuronCores.

Strategy (data-parallel over batch B=256 -> 32 sentences/core):
  - Host: per-sentence edge list -> dense [200,200] adjacency matrix M
    (M[src,dst] += data). Aggregations become dense matmuls:
      in_t  = M^T @ (X W_in + b)   ;  out_t = M @ (X W_out + b)
  - X^T is pre-transposed on host and augmented with a ones-row so the
    biases ride inside the matmul (K = 361).
  - One fused weight matrix wt [361, 1083]:
      [W_in 0:360 | W_out 360:720 | W_loop 720:1080 | gin | gout | gloop]
  - Device per (sentence, row-tile): Z in two PSUM tiles
      zpA [128, 2 banks]: W_in-proj @ bank0, W_out-proj @ bank1
      zpB [128, 1 bank]:  W_loop-proj + the 3 gate columns
    then agg with M/M^T stationary into one 2-bank PSUM tile (gate col
    rides along), and a fused gating epilogue:
      t3 = sigmoid(gloop)*Zloop on eviction (scalar activation scale),
      s1 = (agg_in*sig(gin_agg)) + t3 and s2 = (agg_out*sig)+s1 via
      scalar_tensor_tensor, relu on vector.
  - PE order interleaves main(s) and agg(s-1) at row-tile granularity so
    PSUM-bank reuse never stalls the PE; evictions are spread over
    scalar/vector/gpsimd.
  - Everything bf16 on device (fp32 accumulation in PSUM).
"""
import sys

if "/opt/trn_rl_repo" not in sys.path:
    sys.path.insert(0, "/opt/trn_rl_repo")

from contextlib import ExitStack

import ml_dtypes
import numpy as np

B, NN, EE, DIN, DOUT = 256, 200, 400, 360, 360
NCORES = 8
SPC = B // NCORES          # 32 sentences per core
ROWS = SPC * NN            # 6400 rows per core
KA = DIN + 1               # 361: augmented contraction (ones row for bias)
WCOLS = 3 * DOUT + 3       # 1083 fused weight columns
KCH = [(0, 121), (121, 241), (241, 361)]   # K chunks <= 128
NBF16 = np.dtype(ml_dtypes.bfloat16)

_compiled = None


def _build():
    import concourse.bacc as bacc
    import concourse.mybir as mybir
    from concourse.tile import TileContext

    BF16 = mybir.dt.bfloat16
    F32 = mybir.dt.float32
    AF = mybir.ActivationFunctionType
    OP = mybir.AluOpType

    nc = bacc.Bacc(None, target_bir_lowering=False)
    xt_d = nc.dram_tensor("xt", [KA, ROWS], BF16, kind="ExternalInput")
    wt_d = nc.dram_tensor("wt", [KA, WCOLS], BF16, kind="ExternalInput")
    # per sentence: [mf 0:400 | mb 400:800], each chunked (src rows 0:128 at
    # cols 0:200, src rows 128:200 at cols 200:400; free dim = dst)
    madj_d = nc.dram_tensor("madj", [SPC, 128, 800], BF16, kind="ExternalInput")
    out_d = nc.dram_tensor("out", [SPC, 128, 2 * DOUT], BF16, kind="ExternalOutput")

    with TileContext(nc) as tc, ExitStack() as ctx:
        cpool = ctx.enter_context(tc.tile_pool(name="cpool", bufs=1))
        mpool = ctx.enter_context(tc.tile_pool(name="mpool", bufs=4))
        zspool = ctx.enter_context(tc.tile_pool(name="zspool", bufs=4))
        tpool = ctx.enter_context(tc.tile_pool(name="tpool", bufs=4))
        opool = ctx.enter_context(tc.tile_pool(name="opool", bufs=4))
        zpa = ctx.enter_context(tc.tile_pool(name="zpa", bufs=2, space="PSUM"))
        zpb = ctx.enter_context(tc.tile_pool(name="zpb", bufs=2, space="PSUM"))
        app = ctx.enter_context(tc.tile_pool(name="app", bufs=1, space="PSUM"))

        # ---- resident inputs: weights first (tiny), then X^T with the
        # first sentences' columns landing first ----
        wt_tiles = []
        wt_eng = [nc.scalar, nc.gpsimd, nc.sync]
        for kc, (k0, k1) in enumerate(KCH):
            t = cpool.tile([k1 - k0, WCOLS], BF16, name=f"wt{kc}")
            wt_eng[kc].dma_start(out=t, in_=wt_d[k0:k1, :])
            wt_tiles.append(t)
        xt_tiles = [cpool.tile([k1 - k0, ROWS], BF16, name=f"xt{kc}")
                    for kc, (k0, k1) in enumerate(KCH)]
        first_eng = [nc.scalar, nc.gpsimd, nc.sync]
        for kc, (k0, k1) in enumerate(KCH):
            first_eng[kc].dma_start(out=xt_tiles[kc][:, 0:240], in_=xt_d[k0:k1, 0:240])
        for j in [240, 1040, 1840, 2640, 3440, 4920]:
            w = 800 if j < 3440 else 1480
            for kc, (k0, k1) in enumerate(KCH):
                nc.sync.dma_start(out=xt_tiles[kc][:, j:j + w], in_=xt_d[k0:k1, j:j + w])

        state = {}   # (s, mt) -> per row-tile tensors; s -> madj tile / out tile

        def emit_main(s, mt):
            rows = 128 if mt == 0 else 72
            c0 = s * NN + mt * 128
            if mt == 0:
                madj_t = mpool.tile([128, 800], BF16, tag="madj", name=f"madj{s}")
                nc.gpsimd.dma_start(out=madj_t, in_=madj_d[s])
                state[s] = madj_t
            za = zpa.tile([128, 1024], F32, tag="za", name=f"za{s}_{mt}")
            zb = zpb.tile([128, 512], F32, tag="zb", name=f"zb{s}_{mt}")
            for kc in range(3):
                lt = xt_tiles[kc][:, c0:c0 + rows]
                nc.tensor.matmul(za[0:rows, 0:360], lhsT=lt,
                                 rhs=wt_tiles[kc][:, 0:360],
                                 start=(kc == 0), stop=(kc == 2))
                nc.tensor.matmul(za[0:rows, 512:872], lhsT=lt,
                                 rhs=wt_tiles[kc][:, 360:720],
                                 start=(kc == 0), stop=(kc == 2))
                nc.tensor.matmul(zb[0:rows, 0:363], lhsT=lt,
                                 rhs=wt_tiles[kc][:, 720:1083],
                                 start=(kc == 0), stop=(kc == 2))
            zs_t = zspool.tile([128, 722], BF16, tag="zs", name=f"zs{s}_{mt}")
            sgl = tpool.tile([128, 1], F32, tag="sgl", name=f"sgl{s}_{mt}")
            t3 = tpool.tile([128, 360], BF16, tag="t3", name=f"t3{s}_{mt}")
            # loop gate + gated loop-term eviction (fused scale)
            nc.scalar.activation(sgl[0:rows], zb[0:rows, 362:363], AF.Sigmoid)
            nc.scalar.activation(t3[0:rows], zb[0:rows, 0:360], AF.Copy,
                                 scale=sgl[0:rows, 0:1])
            # evict Z_in/Z_out feats + raw gate cols into agg-rhs layout
            # (gpsimd cannot read PSUM: split across scalar and vector)
            zs_v = zs_t[0:rows].rearrange("p (a c) -> p a c", a=2, c=361)
            nc.scalar.copy(zs_t[0:rows, 0:360], za[0:rows, 0:360])
            nc.vector.tensor_copy(zs_t[0:rows, 361:721], za[0:rows, 512:872])
            nc.scalar.copy(zs_v[:, :, 360], zb[0:rows, 360:362])
            state[(s, mt)] = (zs_t, t3)

        def emit_agg(s, mt):
            rows = 128 if mt == 0 else 72
            d0 = mt * 128
            madj_t = state[s]
            zs_lo, _ = state[(s, 0)]
            zs_hi, _ = state[(s, 1)]
            _, t3 = state[(s, mt)]
            if mt == 1:
                del state[(s, 0)], state[(s, 1)], state[s]
            ap = app.tile([128, 1024], F32, tag="ap", name=f"ap{s}_{mt}")
            # in-arcs: lhsT = mf columns (dst slice), rhs = [Z_in|gin]
            nc.tensor.matmul(ap[0:rows, 0:361],
                             lhsT=madj_t[0:128, d0:d0 + rows],
                             rhs=zs_lo[0:128, 0:361], start=True, stop=False)
            nc.tensor.matmul(ap[0:rows, 0:361],
                             lhsT=madj_t[0:72, 200 + d0:200 + d0 + rows],
                             rhs=zs_hi[0:72, 0:361], start=False, stop=True)
            # out-arcs: lhsT = mb columns, rhs = [Z_out|gout]
            nc.tensor.matmul(ap[0:rows, 512:873],
                             lhsT=madj_t[0:128, 400 + d0:400 + d0 + rows],
                             rhs=zs_lo[0:128, 361:722], start=True, stop=False)
            nc.tensor.matmul(ap[0:rows, 512:873],
                             lhsT=madj_t[0:72, 600 + d0:600 + d0 + rows],
                             rhs=zs_hi[0:72, 361:722], start=False, stop=True)
            # gating epilogue
            sg2 = tpool.tile([128, 2], F32, tag="sg2", name=f"sg2{s}_{mt}")
            ap_v = ap[0:rows].rearrange("p (a c) -> p a c", a=2, c=512)
            nc.scalar.activation(sg2[0:rows], ap_v[:, :, 360], AF.Sigmoid)
            s1 = tpool.tile([128, 360], BF16, tag="s1", name=f"s1{s}_{mt}")
            s2 = tpool.tile([128, 360], BF16, tag="s2", name=f"s2{s}_{mt}")
            nc.vector.scalar_tensor_tensor(
                s1[0:rows], ap[0:rows, 0:360], sg2[0:rows, 0:1], t3[0:rows],
                OP.mult, OP.add)
            nc.vector.scalar_tensor_tensor(
                s2[0:rows], ap[0:rows, 512:872], sg2[0:rows, 1:2], s1[0:rows],
                OP.mult, OP.add)
            ot = opool.tile([128, DOUT], BF16, tag="ot", name=f"ot{s}_{mt}")
            nc.vector.tensor_scalar_max(ot[0:rows], s2[0:rows], 0.0)
            eng = nc.sync if s == SPC - 1 else nc.gpsimd
            eng.dma_start(out=out_d[s, 0:rows, mt * DOUT:(mt + 1) * DOUT],
                          in_=ot[0:rows])

        # software pipeline: agg runs one sentence behind main, interleaved
        # at row-tile granularity
        for s in range(SPC):
            emit_main(s, 0)
            if s > 0:
                emit_agg(s - 1, 0)
            emit_main(s, 1)
            if s > 0:
                emit_agg(s - 1, 1)
        emit_agg(SPC - 1, 0)
        emit_agg(SPC - 1, 1)

    nc.compile()
    return nc


def _get_compiled():
    global _compiled
    if _compiled is None:
        _compiled = _build()
    return _compiled


def kernel(gcn_in, adj_ind, adj_data, w_in, b_in, w_out, b_out, w_loop,
           w_gin, b_gin, w_gout, b_gout, w_gloop):
    from concourse.bass_utils import run_bass_kernel_spmd

    x = np.asarray(gcn_in, np.float32)           # [B, N, DIN]
    idx = np.asarray(adj_ind)[0]                 # [B, E, 2] int
    dat = np.asarray(adj_data, np.float32)[0]    # [B, E]

    # fused weight matrix with bias row:
    # [W_in | W_out | W_loop | gin | gout | gloop]
    wt = np.zeros((KA, WCOLS), np.float32)
    wt[0:DIN, 0:360] = np.asarray(w_in, np.float32)
    wt[DIN, 0:360] = np.asarray(b_in, np.float32)[0]
    wt[0:DIN, 360:720] = np.asarray(w_out, np.float32)
    wt[DIN, 360:720] = np.asarray(b_out, np.float32)[0]
    wt[0:DIN, 720:1080] = np.asarray(w_loop, np.float32)
    wt[0:DIN, 1080] = np.asarray(w_gin, np.float32)[:, 0]
    wt[DIN, 1080] = np.asarray(b_gin, np.float32)[0]
    wt[0:DIN, 1081] = np.asarray(w_gout, np.float32)[:, 0]
    wt[DIN, 1081] = np.asarray(b_gout, np.float32)[0]
    wt[0:DIN, 1082] = np.asarray(w_gloop, np.float32)[:, 0]
    wt = wt.astype(NBF16)

    # dense per-sentence adjacency matrices
    M = np.zeros((B, NN, NN), np.float32)
    bi = np.broadcast_to(np.arange(B)[:, None], idx.shape[:2])
    np.add.at(M, (bi, idx[:, :, 0].astype(np.int64), idx[:, :, 1].astype(np.int64)), dat)

    def chunked(mm):      # [SPC,200,200] -> [SPC,128,400]: two 128-row chunks side by side
        out = np.zeros((SPC, 128, 2 * NN), np.float32)
        out[:, :, 0:NN] = mm[:, 0:128, :]
        out[:, 0:72, NN:2 * NN] = mm[:, 128:200, :]
        return out

    nc = _get_compiled()
    in_maps = []
    for c in range(NCORES):
        xc = x[c * SPC:(c + 1) * SPC].reshape(ROWS, DIN)
        xt = np.empty((KA, ROWS), np.float32)
        xt[0:DIN] = xc.T
        xt[DIN] = 1.0
        mc = M[c * SPC:(c + 1) * SPC]
        madj = np.concatenate(
            [chunked(mc), chunked(np.ascontiguousarray(mc.transpose(0, 2, 1)))],
            axis=2)
        in_maps.append({
            "xt": np.ascontiguousarray(xt).astype(NBF16),
            "wt": wt,
            "madj": madj.astype(NBF16),
        })

    res = run_bass_kernel_spmd(nc, in_maps, core_ids=list(range(NCORES)))
    kernel.last_results = res
    out = np.empty((B, NN, DOUT), np.float32)
    for c in range(NCORES):
        oc = res.results[c]["out"].astype(np.float32)   # [SPC,128,720]
        oc_s = out[c * SPC:(c + 1) * SPC]               # [SPC,200,360]
        oc_s[:, 0:128, :] = oc[:, :, 0:DOUT]
        oc_s[:, 128:200, :] = oc[:, 0:72, DOUT:2 * DOUT]
    return out
